# revision 26
# baseline (speedup 1.0000x reference)
"""Trainium2 Bass kernel for nn_Attention_83476984365503 (sparse_attention).

Reference computation (per batch element b):
    sp      = spatial[b].reshape(C=2048, HW=196)          # [C, HW]
    pre     = sp.T @ W1s + hidden[b] @ W1h + b1           # [HW, D=1024]
    scores  = tanh(pre) @ W2 + b2                         # [HW]
    attn    = softmax(scores)                             # [HW]
    context = sp @ attn                                   # [C]
Returns (context [B, C], attn [B, HW]).

Strategy: data-parallel over 8 NeuronCores (32 batches each), no collectives.
Per core the dominant GEMM is computed transposed as
    preT[d, (b, hw)] = W1s.T @ RHS       RHS[c, (b, hw)] = spatial[b, c, hw]
in float32r (full PE rate), streaming (b, hw) column-blocks of 2 batches
(392 columns). Per block: GEMM -> fused tanh(+per-batch bias) on ScalarE ->
scores matvec on PE. The softmax / attn-broadcast / context reduction of
block j-1 is issued after block j's PE work (software pipelining) so the PE
never head-of-line blocks on the softmax chain. Context uses the fused
scalar_tensor_tensor multiply+reduce on VectorE; attn broadcast runs on the
GpSimd engine (partition_broadcast).
"""
import numpy as np

B, HID, C, HW = 256, 1024, 2048, 196
NCORES = 8
BL = B // NCORES          # 32 batches per core
BPB = 2                   # batches per column block
NBLK = BL // BPB          # 16 blocks
KT = C // 128             # 16 k-tiles over channels
MT = HID // 128           # 8 m-tiles over hidden dim
HT = HID // 128           # 8 k-tiles over hidden dim (hid @ W1h)

_CACHE = {}


def _build():
    import concourse.bass as bass
    import concourse.tile as tile
    from concourse import bacc, mybir
    from concourse.masks import make_identity

    f32 = mybir.dt.float32
    f32r = mybir.dt.float32r
    bf16 = mybir.dt.bfloat16
    AF = mybir.ActivationFunctionType

    nc = bacc.Bacc("TRN2", target_bir_lowering=False, debug=False,
                   num_devices=NCORES)

    hidden = nc.dram_tensor("hidden", [BL, HID], f32, kind="ExternalInput").ap()
    spatial = nc.dram_tensor("spatial", [C, BL, HW], mybir.dt.bfloat16, kind="ExternalInput").ap()
    w1h = nc.dram_tensor("w1h", [HID, HID], mybir.dt.bfloat16, kind="ExternalInput").ap()
    w1s = nc.dram_tensor("w1s", [C, HID], mybir.dt.bfloat16, kind="ExternalInput").ap()
    b1i = nc.dram_tensor("b1i", [128, MT + 128], f32, kind="ExternalInput").ap()
    w2t = nc.dram_tensor("w2t", [128, MT], mybir.dt.bfloat16, kind="ExternalInput").ap()
    ctx_out = nc.dram_tensor("ctx_out", [BL, C], f32, kind="ExternalOutput").ap()
    attn_out = nc.dram_tensor("attn_out", [BL, HW], f32, kind="ExternalOutput").ap()

    with tile.TileContext(nc) as tc:
        with (
            tc.tile_pool(name="consts", bufs=1) as consts,
            tc.tile_pool(name="w1s_p", bufs=1) as w1s_p,
            tc.tile_pool(name="w1h_p", bufs=2) as w1h_p,
            tc.tile_pool(name="rhs_p", bufs=4) as rhs_p,
            tc.tile_pool(name="tanh_p", bufs=2) as tanh_p,
            tc.tile_pool(name="small_p", bufs=2) as small_p,
            tc.tile_pool(name="ps_gemm", bufs=4, space="PSUM") as ps_gemm,
            tc.tile_pool(name="ps_sc", bufs=2, space="PSUM") as ps_sc,
            tc.tile_pool(name="ps_aux", bufs=2, space="PSUM") as ps_aux,
        ):
            # ---------------- DMA order: unblock the GEMM ASAP -------------
            b1i_sb = consts.tile([128, MT + 128], f32)
            nc.sync.dma_start(b1i_sb[:], b1i)
            b1_sb = b1i_sb[:, :MT]
            ident = b1i_sb[:, MT:]
            hid_sb = consts.tile([BL, HID], f32)
            nc.sync.dma_start(hid_sb[:], hidden)

            # All loads ride one HWDGE queue (FIFO = explicit bandwidth
            # schedule; two queues would split bandwidth 50/50 at packet
            # granularity and starve W1s during ramp-up). W1s goes m-major
            # so GEMM m-tiles unlock early; the first two rhs blocks
            # interleave between W1s m-tiles; later rhs DMAs are naturally
            # delayed by tile-slot reuse.

            def new_rhs(j):
                rhs = rhs_p.tile([128, KT, BPB, HW], bf16, name="rhs", tag="rhs")
                nc.sync.dma_start(
                    rhs[:],
                    spatial[:, j * BPB:(j + 1) * BPB, :]
                    .rearrange("(k p) b h -> p k b h", p=128))
                return rhs

            w1s_sb = w1s_p.tile([128, KT, HID], bf16)

            def dma_w1s_m(m, n=1):
                nc.sync.dma_start(
                    w1s_sb[:, :, m * 128:(m + n) * 128],
                    w1s[:, m * 128:(m + n) * 128]
                    .rearrange("(k p) d -> p k d", p=128))

            dma_w1s_m(0, 2)
            # block 0: per-k rhs DMAs so m0's k-loop starts on first arrival
            rhs0 = rhs_p.tile([128, KT, BPB, HW], bf16, name="rhs", tag="rhs")
            for kg in range(0, KT, 4):
                nc.sync.dma_start(
                    rhs0[:, kg:kg + 4, :, :],
                    spatial[kg * 128:(kg + 4) * 128, 0:BPB, :]
                    .rearrange("(k p) b h -> p k b h", p=128))
            dma_w1s_m(2, 2)
            w2_sb = consts.tile([128, MT], mybir.dt.bfloat16)
            nc.sync.dma_start(w2_sb[:], w2t)
            w1h_tiles = []
            for k in range(HT):
                w1h_t = w1h_p.tile([128, HID], bf16, name="w1h_t", tag="w1h")
                nc.sync.dma_start(w1h_t[:], w1h[k * 128:(k + 1) * 128, :])
                w1h_tiles.append(w1h_t)
            dma_w1s_m(4, 2)
            dma_w1s_m(6, 2)
            rhs1 = new_rhs(1)

            # ---------------- per-batch bias: hb = (hidden @ W1h).T + b1 ---
            # transpose hidden -> hidT [h, b] (8 PE transposes of [32, 128])
            hidT_sb = consts.tile([128, HT, BL], bf16)
            for k in range(HT):
                ps_t = ps_aux.tile([128, BL], f32, tag="aux")
                nc.tensor.matmul(ps_t[:], hid_sb[:, k * 128:(k + 1) * 128],
                                 ident[:BL, :BL], is_transpose=True)
                nc.scalar.activation(hidT_sb[:, k, :], ps_t[:], AF.Identity)

            # hid1 = hidden @ W1h  [BL, HID]  (bf16)
            hid1_sb = consts.tile([BL, HID], f32)
            ps_h1 = [ps_aux.tile([BL, 512], f32, name=f"ps_h1_{c}", tag="aux")
                     for c in range(2)]
            for k in range(HT):
                for chunk in range(2):
                    nc.tensor.matmul(ps_h1[chunk][:], hidT_sb[:, k, :],
                                     w1h_tiles[k][:, chunk * 512:(chunk + 1) * 512],
                                     start=(k == 0), stop=(k == HT - 1))
            for chunk in range(2):
                nc.scalar.activation(hid1_sb[:, chunk * 512:(chunk + 1) * 512],
                                     ps_h1[chunk][:], AF.Identity)

            # hb[d, b] = hid1.T + b1  (8 transposes of [32, 128] + bias add)
            hb_sb = consts.tile([128, MT, BL], f32)
            for m in range(MT):
                ps_t = ps_aux.tile([128, BL], f32, tag="aux")
                nc.tensor.matmul(ps_t[:], hid1_sb[:, m * 128:(m + 1) * 128],
                                 ident[:BL, :BL], is_transpose=True)
                nc.scalar.activation(hb_sb[:, m, :], ps_t[:], AF.Identity,
                                     bias=b1_sb[:, m:m + 1])

            # context accumulator ctxT[c_lo, (k, b)]
            ctxT = consts.tile([128, KT * BL], f32)
            ctx_sb = consts.tile([BL, C], f32)

            # ---------------- pipelined epilogue ---------------------------
            def epilogue(j, ps_score, rhs, final=False):
                # softmax over hw within each batch segment (partition 0).
                # No max subtraction: |scores| <= ||W2||_1 ~ 9 (tanh bounded),
                # so exp cannot overflow and softmax is shift-invariant.
                e_sb = small_p.tile([1, BPB, HW], f32, name="e", tag="e")
                nc.scalar.activation(e_sb[:], ps_score[:], AF.Exp)
                sm = small_p.tile([1, BPB, 1], f32, name="sm", tag="sm")
                nc.vector.reduce_sum(sm[:], e_sb[:], axis=mybir.AxisListType.X)
                rec = small_p.tile([1, BPB, 1], f32, name="rec", tag="rec")
                nc.vector.reciprocal(rec[:], sm[:])
                attn_sb = small_p.tile([1, BPB, HW], f32, name="attn", tag="attn")
                nc.vector.tensor_mul(attn_sb[:], e_sb[:],
                                     rec[:].broadcast_to([1, BPB, HW]))
                nc.sync.dma_start(attn_out[j * BPB:(j + 1) * BPB, :], attn_sb[:])

                # broadcast attn to 128 partitions (GpSimd engine)
                att_bc = small_p.tile([128, BPB, HW], f32, name="att_bc",
                                      tag="att_bc")
                nc.gpsimd.partition_broadcast(att_bc[:], attn_sb[:])

                # context: ctxT[c, (k, col)] = sum_hw rhs * attn (fused on DVE)
                junk = small_p.tile([128, HW], f32, name="junk", tag="junk")
                for k in range(KT):
                    for bi in range(BPB):
                        col = k * BL + j * BPB + bi
                        nc.vector.scalar_tensor_tensor(
                            out=junk[:],
                            in0=rhs[:, k, bi, :],
                            scalar=1.0,
                            in1=att_bc[:, bi, :],
                            op0=mybir.AluOpType.mult,
                            op1=mybir.AluOpType.mult,
                            accum_out=ctxT[:, col:col + 1])
                    if final:
                        # ctxT row-k complete: transpose + stage + write out
                        # in quarters (single staging tile, no slot churn)
                        ps_t2 = ps_aux.tile([BL, 128], f32, tag="aux")
                        nc.tensor.matmul(ps_t2[:], ctxT[:, k * BL:(k + 1) * BL],
                                         ident[:, :], is_transpose=True)
                        nc.scalar.activation(
                            ctx_sb[:, k * 128:(k + 1) * 128], ps_t2[:],
                            AF.Identity)
                        nc.sync.dma_start(
                            ctx_out[:, k * 128:(k + 1) * 128],
                            ctx_sb[:, k * 128:(k + 1) * 128])

            # ---------------- main loop over column blocks -----------------
            pending = None
            for j in range(NBLK):
                rhs = (rhs0, rhs1)[j] if j < 2 else new_rhs(j)

                tanh_sb = tanh_p.tile([128, MT, BPB, HW], mybir.dt.bfloat16, name="tanh_sb",
                                      tag="tanh")
                for m in range(MT):
                    ps_pre = ps_gemm.tile([128, BPB, HW], f32, name="ps_pre",
                                          tag="gemm")
                    for k in range(KT):
                        nc.tensor.matmul(
                            ps_pre[:],
                            w1s_sb[:, k, m * 128:(m + 1) * 128],
                            rhs[:, k, :, :],
                            start=(k == 0), stop=(k == KT - 1))
                    for bi in range(BPB):
                        nc.scalar.activation(
                            tanh_sb[:, m, bi, :], ps_pre[:, bi, :], AF.Tanh,
                            bias=hb_sb[:, m, j * BPB + bi:j * BPB + bi + 1])

                # scores[(b, hw)] = sum_d tanh * w2
                ps_score = ps_sc.tile([1, BPB, HW], f32, name="ps_score",
                                      tag="score")
                for m in range(MT):
                    nc.tensor.matmul(ps_score[:], w2_sb[:, m:m + 1],
                                     tanh_sb[:, m, :, :],
                                     start=(m == 0), stop=(m == MT - 1))

                if pending is not None:
                    epilogue(*pending)
                pending = (j, ps_score, rhs)
            epilogue(*pending, final=True)

    nc.compile()
    return nc


def _get_nc():
    if "nc" not in _CACHE:
        _CACHE["nc"] = _build()
    return _CACHE["nc"]


def kernel(hidden, spatial_features, W1, b1, W2, b2):
    from concourse.bass_utils import run_bass_kernel_spmd

    hidden = np.asarray(hidden, dtype=np.float32)
    spatial = np.asarray(spatial_features, dtype=np.float32).reshape(B, C, HW)
    import ml_dtypes
    spatial_bf = spatial.astype(ml_dtypes.bfloat16)
    W1 = np.asarray(W1, dtype=np.float32)
    b1 = np.asarray(b1, dtype=np.float32)
    W2 = np.asarray(W2, dtype=np.float32)

    import ml_dtypes
    w1h = np.ascontiguousarray(W1[:HID].astype(ml_dtypes.bfloat16))
    w1s = np.ascontiguousarray(W1[HID:].astype(ml_dtypes.bfloat16))
    b1i = np.ascontiguousarray(np.concatenate(
        [b1.reshape(MT, 128).T, np.eye(128, dtype=np.float32)], axis=1))
    w2t = np.ascontiguousarray(W2[:, 0].reshape(MT, 128).T.astype(ml_dtypes.bfloat16))

    in_maps = [
        {
            "hidden": np.ascontiguousarray(hidden[i * BL:(i + 1) * BL]),
            "spatial": np.ascontiguousarray(
                spatial_bf[i * BL:(i + 1) * BL].transpose(1, 0, 2)),
            "w1h": w1h, "w1s": w1s, "b1i": b1i, "w2t": w2t,
        }
        for i in range(NCORES)
    ]
    nc = _get_nc()
    res = run_bass_kernel_spmd(nc, in_maps, core_ids=list(range(NCORES)))
    ctx = np.concatenate([res.results[i]["ctx_out"] for i in range(NCORES)], axis=0)
    attn = np.concatenate([res.results[i]["attn_out"] for i in range(NCORES)], axis=0)
    return ctx, attn


# revision 27
# speedup vs baseline: 1.0067x; 1.0067x over previous
"""Trainium2 Bass kernel for nn_Attention_83476984365503 (sparse_attention).

Reference computation (per batch element b):
    sp      = spatial[b].reshape(C=2048, HW=196)          # [C, HW]
    pre     = sp.T @ W1s + hidden[b] @ W1h + b1           # [HW, D=1024]
    scores  = tanh(pre) @ W2 + b2                         # [HW]
    attn    = softmax(scores)                             # [HW]
    context = sp @ attn                                   # [C]
Returns (context [B, C], attn [B, HW]).

Strategy: data-parallel over 8 NeuronCores (32 batches each), no collectives.
Per core the dominant GEMM is computed transposed as
    preT[d, (b, hw)] = W1s.T @ RHS       RHS[c, (b, hw)] = spatial[b, c, hw]
in float32r (full PE rate), streaming (b, hw) column-blocks of 2 batches
(392 columns). Per block: GEMM -> fused tanh(+per-batch bias) on ScalarE ->
scores matvec on PE. The softmax / attn-broadcast / context reduction of
block j-1 is issued after block j's PE work (software pipelining) so the PE
never head-of-line blocks on the softmax chain. Context uses the fused
scalar_tensor_tensor multiply+reduce on VectorE; attn broadcast runs on the
GpSimd engine (partition_broadcast).
"""
import numpy as np

B, HID, C, HW = 256, 1024, 2048, 196
NCORES = 8
BL = B // NCORES          # 32 batches per core
BPB = 2                   # batches per column block
NBLK = BL // BPB          # 16 blocks
KT = C // 128             # 16 k-tiles over channels
MT = HID // 128           # 8 m-tiles over hidden dim
HT = HID // 128           # 8 k-tiles over hidden dim (hid @ W1h)

_CACHE = {}


def _build():
    import concourse.bass as bass
    import concourse.tile as tile
    from concourse import bacc, mybir
    from concourse.masks import make_identity

    f32 = mybir.dt.float32
    f32r = mybir.dt.float32r
    bf16 = mybir.dt.bfloat16
    AF = mybir.ActivationFunctionType

    nc = bacc.Bacc("TRN2", target_bir_lowering=False, debug=False,
                   num_devices=NCORES)

    hidden = nc.dram_tensor("hidden", [BL, HID], f32, kind="ExternalInput").ap()
    spatial = nc.dram_tensor("spatial", [C, BL, HW], mybir.dt.bfloat16, kind="ExternalInput").ap()
    w1h = nc.dram_tensor("w1h", [HID, HID], mybir.dt.bfloat16, kind="ExternalInput").ap()
    w1s = nc.dram_tensor("w1s", [C, HID], mybir.dt.bfloat16, kind="ExternalInput").ap()
    b1i = nc.dram_tensor("b1i", [128, MT + 128], f32, kind="ExternalInput").ap()
    w2t = nc.dram_tensor("w2t", [128, MT], mybir.dt.bfloat16, kind="ExternalInput").ap()
    ctx_out = nc.dram_tensor("ctx_out", [BL, C], f32, kind="ExternalOutput").ap()
    attn_out = nc.dram_tensor("attn_out", [BL, HW], f32, kind="ExternalOutput").ap()

    with tile.TileContext(nc) as tc:
        with (
            tc.tile_pool(name="consts", bufs=1) as consts,
            tc.tile_pool(name="w1s_p", bufs=1) as w1s_p,
            tc.tile_pool(name="w1h_p", bufs=2) as w1h_p,
            tc.tile_pool(name="rhs_p", bufs=4) as rhs_p,
            tc.tile_pool(name="tanh_p", bufs=2) as tanh_p,
            tc.tile_pool(name="small_p", bufs=2) as small_p,
            tc.tile_pool(name="ps_gemm", bufs=4, space="PSUM") as ps_gemm,
            tc.tile_pool(name="ps_sc", bufs=2, space="PSUM") as ps_sc,
            tc.tile_pool(name="ps_aux", bufs=2, space="PSUM") as ps_aux,
        ):
            # ---------------- DMA order: unblock the GEMM ASAP -------------
            b1i_sb = consts.tile([128, MT + 128], f32)
            nc.sync.dma_start(b1i_sb[:], b1i)
            b1_sb = b1i_sb[:, :MT]
            ident = b1i_sb[:, MT:]
            w2_sb = consts.tile([128, MT], mybir.dt.bfloat16)
            nc.sync.dma_start(w2_sb[:], w2t)
            hid_sb = consts.tile([BL, HID], f32)
            nc.sync.dma_start(hid_sb[:], hidden)

            # All loads ride one HWDGE queue (FIFO = explicit bandwidth
            # schedule; two queues would split bandwidth 50/50 at packet
            # granularity and starve W1s during ramp-up). W1s goes m-major
            # so GEMM m-tiles unlock early; the first two rhs blocks
            # interleave between W1s m-tiles; later rhs DMAs are naturally
            # delayed by tile-slot reuse.

            def new_rhs(j):
                rhs = rhs_p.tile([128, KT, BPB, HW], bf16, name="rhs", tag="rhs")
                nc.sync.dma_start(
                    rhs[:],
                    spatial[:, j * BPB:(j + 1) * BPB, :]
                    .rearrange("(k p) b h -> p k b h", p=128))
                return rhs

            w1s_sb = w1s_p.tile([128, KT, HID], bf16)

            def dma_w1s_m(m, n=1):
                nc.sync.dma_start(
                    w1s_sb[:, :, m * 128:(m + n) * 128],
                    w1s[:, m * 128:(m + n) * 128]
                    .rearrange("(k p) d -> p k d", p=128))

            dma_w1s_m(0, 2)
            # block 0: per-k rhs DMAs so m0's k-loop starts on first arrival
            rhs0 = rhs_p.tile([128, KT, BPB, HW], bf16, name="rhs", tag="rhs")
            for kg in range(0, KT, 4):
                nc.sync.dma_start(
                    rhs0[:, kg:kg + 4, :, :],
                    spatial[kg * 128:(kg + 4) * 128, 0:BPB, :]
                    .rearrange("(k p) b h -> p k b h", p=128))
            w1h_tiles = []
            for k in range(HT):
                w1h_t = w1h_p.tile([128, HID], bf16, name="w1h_t", tag="w1h")
                nc.sync.dma_start(w1h_t[:], w1h[k * 128:(k + 1) * 128, :])
                w1h_tiles.append(w1h_t)
            dma_w1s_m(2, 2)
            rhs1 = new_rhs(1)
            dma_w1s_m(4, 2)
            dma_w1s_m(6, 2)

            # ---------------- per-batch bias: hb = (hidden @ W1h).T + b1 ---
            # transpose hidden -> hidT [h, b] (8 PE transposes of [32, 128])
            hidT_sb = consts.tile([128, HT, BL], bf16)
            for k in range(HT):
                ps_t = ps_aux.tile([128, BL], f32, tag="aux")
                nc.tensor.matmul(ps_t[:], hid_sb[:, k * 128:(k + 1) * 128],
                                 ident[:BL, :BL], is_transpose=True)
                nc.scalar.activation(hidT_sb[:, k, :], ps_t[:], AF.Identity)

            # hid1 = hidden @ W1h  [BL, HID]  (bf16)
            hid1_sb = consts.tile([BL, HID], f32)
            ps_h1 = [ps_aux.tile([BL, 512], f32, name=f"ps_h1_{c}", tag="aux")
                     for c in range(2)]
            for k in range(HT):
                for chunk in range(2):
                    nc.tensor.matmul(ps_h1[chunk][:], hidT_sb[:, k, :],
                                     w1h_tiles[k][:, chunk * 512:(chunk + 1) * 512],
                                     start=(k == 0), stop=(k == HT - 1))
            for chunk in range(2):
                nc.scalar.activation(hid1_sb[:, chunk * 512:(chunk + 1) * 512],
                                     ps_h1[chunk][:], AF.Identity)

            # hb[d, b] = hid1.T + b1  (8 transposes of [32, 128] + bias add)
            hb_sb = consts.tile([128, MT, BL], f32)
            for m in range(MT):
                ps_t = ps_aux.tile([128, BL], f32, tag="aux")
                nc.tensor.matmul(ps_t[:], hid1_sb[:, m * 128:(m + 1) * 128],
                                 ident[:BL, :BL], is_transpose=True)
                nc.scalar.activation(hb_sb[:, m, :], ps_t[:], AF.Identity,
                                     bias=b1_sb[:, m:m + 1])

            # context accumulator ctxT[c_lo, (k, b)]
            ctxT = consts.tile([128, KT * BL], f32)
            ctx_sb = consts.tile([BL, C], f32)

            # ---------------- pipelined epilogue ---------------------------
            def epilogue(j, ps_score, rhs, final=False):
                # softmax over hw within each batch segment (partition 0).
                # No max subtraction: |scores| <= ||W2||_1 ~ 9 (tanh bounded),
                # so exp cannot overflow and softmax is shift-invariant.
                e_sb = small_p.tile([1, BPB, HW], f32, name="e", tag="e")
                nc.scalar.activation(e_sb[:], ps_score[:], AF.Exp)
                sm = small_p.tile([1, BPB, 1], f32, name="sm", tag="sm")
                nc.vector.reduce_sum(sm[:], e_sb[:], axis=mybir.AxisListType.X)
                rec = small_p.tile([1, BPB, 1], f32, name="rec", tag="rec")
                nc.vector.reciprocal(rec[:], sm[:])
                attn_sb = small_p.tile([1, BPB, HW], f32, name="attn", tag="attn")
                nc.vector.tensor_mul(attn_sb[:], e_sb[:],
                                     rec[:].broadcast_to([1, BPB, HW]))
                nc.sync.dma_start(attn_out[j * BPB:(j + 1) * BPB, :], attn_sb[:])

                # broadcast attn to 128 partitions (GpSimd engine)
                att_bc = small_p.tile([128, BPB, HW], f32, name="att_bc",
                                      tag="att_bc")
                nc.gpsimd.partition_broadcast(att_bc[:], attn_sb[:])

                # context: ctxT[c, (k, col)] = sum_hw rhs * attn (fused on DVE)
                junk = small_p.tile([128, HW], f32, name="junk", tag="junk")
                for k in range(KT):
                    for bi in range(BPB):
                        col = k * BL + j * BPB + bi
                        nc.vector.scalar_tensor_tensor(
                            out=junk[:],
                            in0=rhs[:, k, bi, :],
                            scalar=1.0,
                            in1=att_bc[:, bi, :],
                            op0=mybir.AluOpType.mult,
                            op1=mybir.AluOpType.mult,
                            accum_out=ctxT[:, col:col + 1])
                    if final:
                        # ctxT row-k complete: transpose + stage + write out
                        # in quarters (single staging tile, no slot churn)
                        ps_t2 = ps_aux.tile([BL, 128], f32, tag="aux")
                        nc.tensor.matmul(ps_t2[:], ctxT[:, k * BL:(k + 1) * BL],
                                         ident[:, :], is_transpose=True)
                        nc.scalar.activation(
                            ctx_sb[:, k * 128:(k + 1) * 128], ps_t2[:],
                            AF.Identity)
                        nc.sync.dma_start(
                            ctx_out[:, k * 128:(k + 1) * 128],
                            ctx_sb[:, k * 128:(k + 1) * 128])

            # ---------------- main loop over column blocks -----------------
            pending = None
            for j in range(NBLK):
                rhs = (rhs0, rhs1)[j] if j < 2 else new_rhs(j)

                tanh_sb = tanh_p.tile([128, MT, BPB, HW], mybir.dt.bfloat16, name="tanh_sb",
                                      tag="tanh")
                for m in range(MT):
                    ps_pre = ps_gemm.tile([128, BPB, HW], f32, name="ps_pre",
                                          tag="gemm")
                    for k in range(KT):
                        nc.tensor.matmul(
                            ps_pre[:],
                            w1s_sb[:, k, m * 128:(m + 1) * 128],
                            rhs[:, k, :, :],
                            start=(k == 0), stop=(k == KT - 1))
                    for bi in range(BPB):
                        nc.scalar.activation(
                            tanh_sb[:, m, bi, :], ps_pre[:, bi, :], AF.Tanh,
                            bias=hb_sb[:, m, j * BPB + bi:j * BPB + bi + 1])

                # scores[(b, hw)] = sum_d tanh * w2
                ps_score = ps_sc.tile([1, BPB, HW], f32, name="ps_score",
                                      tag="score")
                for m in range(MT):
                    nc.tensor.matmul(ps_score[:], w2_sb[:, m:m + 1],
                                     tanh_sb[:, m, :, :],
                                     start=(m == 0), stop=(m == MT - 1))

                if pending is not None:
                    epilogue(*pending)
                pending = (j, ps_score, rhs)
            epilogue(*pending, final=True)

    nc.compile()
    return nc


def _get_nc():
    if "nc" not in _CACHE:
        _CACHE["nc"] = _build()
    return _CACHE["nc"]


def kernel(hidden, spatial_features, W1, b1, W2, b2):
    from concourse.bass_utils import run_bass_kernel_spmd

    hidden = np.asarray(hidden, dtype=np.float32)
    spatial = np.asarray(spatial_features, dtype=np.float32).reshape(B, C, HW)
    import ml_dtypes
    spatial_bf = spatial.astype(ml_dtypes.bfloat16)
    W1 = np.asarray(W1, dtype=np.float32)
    b1 = np.asarray(b1, dtype=np.float32)
    W2 = np.asarray(W2, dtype=np.float32)

    import ml_dtypes
    w1h = np.ascontiguousarray(W1[:HID].astype(ml_dtypes.bfloat16))
    w1s = np.ascontiguousarray(W1[HID:].astype(ml_dtypes.bfloat16))
    b1i = np.ascontiguousarray(np.concatenate(
        [b1.reshape(MT, 128).T, np.eye(128, dtype=np.float32)], axis=1))
    w2t = np.ascontiguousarray(W2[:, 0].reshape(MT, 128).T.astype(ml_dtypes.bfloat16))

    in_maps = [
        {
            "hidden": np.ascontiguousarray(hidden[i * BL:(i + 1) * BL]),
            "spatial": np.ascontiguousarray(
                spatial_bf[i * BL:(i + 1) * BL].transpose(1, 0, 2)),
            "w1h": w1h, "w1s": w1s, "b1i": b1i, "w2t": w2t,
        }
        for i in range(NCORES)
    ]
    nc = _get_nc()
    res = run_bass_kernel_spmd(nc, in_maps, core_ids=list(range(NCORES)))
    ctx = np.concatenate([res.results[i]["ctx_out"] for i in range(NCORES)], axis=0)
    attn = np.concatenate([res.results[i]["attn_out"] for i in range(NCORES)], axis=0)
    return ctx, attn


# revision 28
# speedup vs baseline: 1.0071x; 1.0004x over previous
"""Trainium2 Bass kernel for nn_Attention_83476984365503 (sparse_attention).

Reference computation (per batch element b):
    sp      = spatial[b].reshape(C=2048, HW=196)          # [C, HW]
    pre     = sp.T @ W1s + hidden[b] @ W1h + b1           # [HW, D=1024]
    scores  = tanh(pre) @ W2 + b2                         # [HW]
    attn    = softmax(scores)                             # [HW]
    context = sp @ attn                                   # [C]
Returns (context [B, C], attn [B, HW]).

Strategy: data-parallel over 8 NeuronCores (32 batches each), no collectives.
Per core the dominant GEMM is computed transposed as
    preT[d, (b, hw)] = W1s.T @ RHS       RHS[c, (b, hw)] = spatial[b, c, hw]
in float32r (full PE rate), streaming (b, hw) column-blocks of 2 batches
(392 columns). Per block: GEMM -> fused tanh(+per-batch bias) on ScalarE ->
scores matvec on PE. The softmax / attn-broadcast / context reduction of
block j-1 is issued after block j's PE work (software pipelining) so the PE
never head-of-line blocks on the softmax chain. Context uses the fused
scalar_tensor_tensor multiply+reduce on VectorE; attn broadcast runs on the
GpSimd engine (partition_broadcast).
"""
import numpy as np

B, HID, C, HW = 256, 1024, 2048, 196
NCORES = 8
BL = B // NCORES          # 32 batches per core
BPB = 2                   # batches per column block
NBLK = BL // BPB          # 16 blocks
KT = C // 128             # 16 k-tiles over channels
MT = HID // 128           # 8 m-tiles over hidden dim
HT = HID // 128           # 8 k-tiles over hidden dim (hid @ W1h)

_CACHE = {}


def _build():
    import concourse.bass as bass
    import concourse.tile as tile
    from concourse import bacc, mybir
    from concourse.masks import make_identity

    f32 = mybir.dt.float32
    f32r = mybir.dt.float32r
    bf16 = mybir.dt.bfloat16
    AF = mybir.ActivationFunctionType

    nc = bacc.Bacc("TRN2", target_bir_lowering=False, debug=False,
                   num_devices=NCORES)

    hidden = nc.dram_tensor("hidden", [BL, HID], f32, kind="ExternalInput").ap()
    spatial = nc.dram_tensor("spatial", [C, BL, HW], mybir.dt.bfloat16, kind="ExternalInput").ap()
    w1h = nc.dram_tensor("w1h", [HID, HID], mybir.dt.bfloat16, kind="ExternalInput").ap()
    w1s = nc.dram_tensor("w1s", [C, HID], mybir.dt.bfloat16, kind="ExternalInput").ap()
    b1i = nc.dram_tensor("b1i", [128, MT + 128], f32, kind="ExternalInput").ap()
    w2t = nc.dram_tensor("w2t", [128, MT], mybir.dt.bfloat16, kind="ExternalInput").ap()
    ctx_out = nc.dram_tensor("ctx_out", [BL, C], f32, kind="ExternalOutput").ap()
    attn_out = nc.dram_tensor("attn_out", [BL, HW], f32, kind="ExternalOutput").ap()

    with tile.TileContext(nc) as tc:
        with (
            tc.tile_pool(name="consts", bufs=1) as consts,
            tc.tile_pool(name="w1s_p", bufs=1) as w1s_p,
            tc.tile_pool(name="w1h_p", bufs=2) as w1h_p,
            tc.tile_pool(name="rhs_p", bufs=4) as rhs_p,
            tc.tile_pool(name="tanh_p", bufs=2) as tanh_p,
            tc.tile_pool(name="small_p", bufs=2) as small_p,
            tc.tile_pool(name="ps_gemm", bufs=4, space="PSUM") as ps_gemm,
            tc.tile_pool(name="ps_sc", bufs=2, space="PSUM") as ps_sc,
            tc.tile_pool(name="ps_aux", bufs=2, space="PSUM") as ps_aux,
        ):
            # ---------------- DMA order: unblock the GEMM ASAP -------------
            b1i_sb = consts.tile([128, MT + 128], f32)
            nc.sync.dma_start(b1i_sb[:], b1i)
            b1_sb = b1i_sb[:, :MT]
            ident = b1i_sb[:, MT:]
            w2_sb = consts.tile([128, MT], mybir.dt.bfloat16)
            nc.sync.dma_start(w2_sb[:], w2t)
            hid_sb = consts.tile([BL, HID], f32)
            nc.sync.dma_start(hid_sb[:], hidden)

            # All loads ride one HWDGE queue (FIFO = explicit bandwidth
            # schedule; two queues would split bandwidth 50/50 at packet
            # granularity and starve W1s during ramp-up). W1s goes m-major
            # so GEMM m-tiles unlock early; the first two rhs blocks
            # interleave between W1s m-tiles; later rhs DMAs are naturally
            # delayed by tile-slot reuse.

            def new_rhs(j):
                rhs = rhs_p.tile([128, KT, BPB, HW], bf16, name="rhs", tag="rhs")
                nc.sync.dma_start(
                    rhs[:],
                    spatial[:, j * BPB:(j + 1) * BPB, :]
                    .rearrange("(k p) b h -> p k b h", p=128))
                return rhs

            w1s_sb = w1s_p.tile([128, KT, HID], bf16)

            def dma_w1s_m(m, n=1):
                nc.sync.dma_start(
                    w1s_sb[:, :, m * 128:(m + n) * 128],
                    w1s[:, m * 128:(m + n) * 128]
                    .rearrange("(k p) d -> p k d", p=128))

            dma_w1s_m(0, 2)
            # block 0: per-k rhs DMAs so m0's k-loop starts on first arrival
            rhs0 = rhs_p.tile([128, KT, BPB, HW], bf16, name="rhs", tag="rhs")
            for kg in range(0, KT, 4):
                nc.sync.dma_start(
                    rhs0[:, kg:kg + 4, :, :],
                    spatial[kg * 128:(kg + 4) * 128, 0:BPB, :]
                    .rearrange("(k p) b h -> p k b h", p=128))
            w1h_tiles = []
            for k in range(HT):
                w1h_t = w1h_p.tile([128, HID], bf16, name="w1h_t", tag="w1h")
                nc.sync.dma_start(w1h_t[:], w1h[k * 128:(k + 1) * 128, :])
                w1h_tiles.append(w1h_t)
            dma_w1s_m(2, 2)
            rhs1 = new_rhs(1)
            dma_w1s_m(4, 2)
            dma_w1s_m(6, 2)

            # ---------------- per-batch bias: hb = (hidden @ W1h).T + b1 ---
            # transpose hidden -> hidT [h, b] (8 PE transposes of [32, 128])
            hidT_sb = consts.tile([128, HT, BL], bf16)
            for k in range(HT):
                ps_t = ps_aux.tile([128, BL], f32, tag="aux")
                nc.tensor.matmul(ps_t[:], hid_sb[:, k * 128:(k + 1) * 128],
                                 ident[:BL, :BL], is_transpose=True)
                nc.scalar.activation(hidT_sb[:, k, :], ps_t[:], AF.Identity)

            # hid1 = hidden @ W1h  [BL, HID]  (bf16)
            hid1_sb = consts.tile([BL, HID], f32)
            ps_h1 = [ps_aux.tile([BL, 512], f32, name=f"ps_h1_{c}", tag="aux")
                     for c in range(2)]
            for k in range(HT):
                for chunk in range(2):
                    nc.tensor.matmul(ps_h1[chunk][:], hidT_sb[:, k, :],
                                     w1h_tiles[k][:, chunk * 512:(chunk + 1) * 512],
                                     start=(k == 0), stop=(k == HT - 1))
            for chunk in range(2):
                nc.scalar.activation(hid1_sb[:, chunk * 512:(chunk + 1) * 512],
                                     ps_h1[chunk][:], AF.Identity)

            # hb[d, b] = hid1.T + b1  (8 transposes of [32, 128] + bias add)
            hb_sb = consts.tile([128, MT, BL], f32)
            for m in range(MT):
                ps_t = ps_aux.tile([128, BL], f32, tag="aux")
                nc.tensor.matmul(ps_t[:], hid1_sb[:, m * 128:(m + 1) * 128],
                                 ident[:BL, :BL], is_transpose=True)
                nc.scalar.activation(hb_sb[:, m, :], ps_t[:], AF.Identity,
                                     bias=b1_sb[:, m:m + 1])

            # context accumulator ctxT[c_lo, (k, b)]
            ctxT = consts.tile([128, KT * BL], f32)
            ctx_sb = consts.tile([BL, C], f32)

            # ---------------- pipelined epilogue ---------------------------
            def epilogue(j, ps_score, rhs, final=False):
                # softmax over hw within each batch segment (partition 0).
                # No max subtraction: |scores| <= ||W2||_1 ~ 9 (tanh bounded),
                # so exp cannot overflow and softmax is shift-invariant.
                e_sb = small_p.tile([1, BPB, HW], f32, name="e", tag="e")
                sm = small_p.tile([1, BPB, 1], f32, name="sm", tag="sm")
                # exp + per-segment sum fused on ScalarE via accum_out
                for bi in range(BPB):
                    nc.scalar.activation(e_sb[:, bi, :], ps_score[:, bi, :],
                                         AF.Exp, accum_out=sm[:, bi, :])
                rec = small_p.tile([1, BPB, 1], f32, name="rec", tag="rec")
                nc.vector.reciprocal(rec[:], sm[:])
                attn_sb = small_p.tile([1, BPB, HW], f32, name="attn", tag="attn")
                nc.vector.tensor_mul(attn_sb[:], e_sb[:],
                                     rec[:].broadcast_to([1, BPB, HW]))
                nc.sync.dma_start(attn_out[j * BPB:(j + 1) * BPB, :], attn_sb[:])

                # broadcast attn to 128 partitions (GpSimd engine)
                att_bc = small_p.tile([128, BPB, HW], f32, name="att_bc",
                                      tag="att_bc")
                nc.gpsimd.partition_broadcast(att_bc[:], attn_sb[:])

                # context: ctxT[c, (k, col)] = sum_hw rhs * attn (fused on DVE)
                junk = small_p.tile([128, HW], f32, name="junk", tag="junk")
                for k in range(KT):
                    for bi in range(BPB):
                        col = k * BL + j * BPB + bi
                        nc.vector.scalar_tensor_tensor(
                            out=junk[:],
                            in0=rhs[:, k, bi, :],
                            scalar=1.0,
                            in1=att_bc[:, bi, :],
                            op0=mybir.AluOpType.mult,
                            op1=mybir.AluOpType.mult,
                            accum_out=ctxT[:, col:col + 1])
                    if final:
                        # ctxT row-k complete: transpose + stage + write out
                        # in quarters (single staging tile, no slot churn)
                        ps_t2 = ps_aux.tile([BL, 128], f32, tag="aux")
                        nc.tensor.matmul(ps_t2[:], ctxT[:, k * BL:(k + 1) * BL],
                                         ident[:, :], is_transpose=True)
                        nc.scalar.activation(
                            ctx_sb[:, k * 128:(k + 1) * 128], ps_t2[:],
                            AF.Identity)
                        nc.sync.dma_start(
                            ctx_out[:, k * 128:(k + 1) * 128],
                            ctx_sb[:, k * 128:(k + 1) * 128])

            # ---------------- main loop over column blocks -----------------
            pending = None
            for j in range(NBLK):
                rhs = (rhs0, rhs1)[j] if j < 2 else new_rhs(j)

                tanh_sb = tanh_p.tile([128, MT, BPB, HW], mybir.dt.bfloat16, name="tanh_sb",
                                      tag="tanh")
                for m in range(MT):
                    ps_pre = ps_gemm.tile([128, BPB, HW], f32, name="ps_pre",
                                          tag="gemm")
                    for k in range(KT):
                        nc.tensor.matmul(
                            ps_pre[:],
                            w1s_sb[:, k, m * 128:(m + 1) * 128],
                            rhs[:, k, :, :],
                            start=(k == 0), stop=(k == KT - 1))
                    for bi in range(BPB):
                        nc.scalar.activation(
                            tanh_sb[:, m, bi, :], ps_pre[:, bi, :], AF.Tanh,
                            bias=hb_sb[:, m, j * BPB + bi:j * BPB + bi + 1])

                # scores[(b, hw)] = sum_d tanh * w2
                ps_score = ps_sc.tile([1, BPB, HW], f32, name="ps_score",
                                      tag="score")
                for m in range(MT):
                    nc.tensor.matmul(ps_score[:], w2_sb[:, m:m + 1],
                                     tanh_sb[:, m, :, :],
                                     start=(m == 0), stop=(m == MT - 1))

                if pending is not None:
                    epilogue(*pending)
                pending = (j, ps_score, rhs)
            epilogue(*pending, final=True)

    nc.compile()
    return nc


def _get_nc():
    if "nc" not in _CACHE:
        _CACHE["nc"] = _build()
    return _CACHE["nc"]


def kernel(hidden, spatial_features, W1, b1, W2, b2):
    from concourse.bass_utils import run_bass_kernel_spmd

    hidden = np.asarray(hidden, dtype=np.float32)
    spatial = np.asarray(spatial_features, dtype=np.float32).reshape(B, C, HW)
    import ml_dtypes
    spatial_bf = spatial.astype(ml_dtypes.bfloat16)
    W1 = np.asarray(W1, dtype=np.float32)
    b1 = np.asarray(b1, dtype=np.float32)
    W2 = np.asarray(W2, dtype=np.float32)

    import ml_dtypes
    w1h = np.ascontiguousarray(W1[:HID].astype(ml_dtypes.bfloat16))
    w1s = np.ascontiguousarray(W1[HID:].astype(ml_dtypes.bfloat16))
    b1i = np.ascontiguousarray(np.concatenate(
        [b1.reshape(MT, 128).T, np.eye(128, dtype=np.float32)], axis=1))
    w2t = np.ascontiguousarray(W2[:, 0].reshape(MT, 128).T.astype(ml_dtypes.bfloat16))

    in_maps = [
        {
            "hidden": np.ascontiguousarray(hidden[i * BL:(i + 1) * BL]),
            "spatial": np.ascontiguousarray(
                spatial_bf[i * BL:(i + 1) * BL].transpose(1, 0, 2)),
            "w1h": w1h, "w1s": w1s, "b1i": b1i, "w2t": w2t,
        }
        for i in range(NCORES)
    ]
    nc = _get_nc()
    res = run_bass_kernel_spmd(nc, in_maps, core_ids=list(range(NCORES)))
    ctx = np.concatenate([res.results[i]["ctx_out"] for i in range(NCORES)], axis=0)
    attn = np.concatenate([res.results[i]["attn_out"] for i in range(NCORES)], axis=0)
    return ctx, attn


# revision 29
# speedup vs baseline: 1.0085x; 1.0014x over previous
"""Trainium2 Bass kernel for nn_Attention_83476984365503 (sparse_attention).

Reference computation (per batch element b):
    sp      = spatial[b].reshape(C=2048, HW=196)          # [C, HW]
    pre     = sp.T @ W1s + hidden[b] @ W1h + b1           # [HW, D=1024]
    scores  = tanh(pre) @ W2 + b2                         # [HW]
    attn    = softmax(scores)                             # [HW]
    context = sp @ attn                                   # [C]
Returns (context [B, C], attn [B, HW]).

Strategy: data-parallel over 8 NeuronCores (32 batches each), no collectives.
Per core the dominant GEMM is computed transposed as
    preT[d, (b, hw)] = W1s.T @ RHS       RHS[c, (b, hw)] = spatial[b, c, hw]
in float32r (full PE rate), streaming (b, hw) column-blocks of 2 batches
(392 columns). Per block: GEMM -> fused tanh(+per-batch bias) on ScalarE ->
scores matvec on PE. The softmax / attn-broadcast / context reduction of
block j-1 is issued after block j's PE work (software pipelining) so the PE
never head-of-line blocks on the softmax chain. Context uses the fused
scalar_tensor_tensor multiply+reduce on VectorE; attn broadcast runs on the
GpSimd engine (partition_broadcast).
"""
import numpy as np

B, HID, C, HW = 256, 1024, 2048, 196
NCORES = 8
BL = B // NCORES          # 32 batches per core
BPB = 2                   # batches per column block
NBLK = BL // BPB          # 16 blocks
KT = C // 128             # 16 k-tiles over channels
MT = HID // 128           # 8 m-tiles over hidden dim
HT = HID // 128           # 8 k-tiles over hidden dim (hid @ W1h)

_CACHE = {}


def _build():
    import concourse.bass as bass
    import concourse.tile as tile
    from concourse import bacc, mybir
    from concourse.masks import make_identity

    f32 = mybir.dt.float32
    f32r = mybir.dt.float32r
    bf16 = mybir.dt.bfloat16
    AF = mybir.ActivationFunctionType

    nc = bacc.Bacc("TRN2", target_bir_lowering=False, debug=False,
                   num_devices=NCORES)

    hidden = nc.dram_tensor("hidden", [BL, HID], f32, kind="ExternalInput").ap()
    spatial = nc.dram_tensor("spatial", [C, BL, HW], mybir.dt.bfloat16, kind="ExternalInput").ap()
    w1h = nc.dram_tensor("w1h", [HID, HID], mybir.dt.bfloat16, kind="ExternalInput").ap()
    w1s = nc.dram_tensor("w1s", [C, HID], mybir.dt.bfloat16, kind="ExternalInput").ap()
    b1i = nc.dram_tensor("b1i", [128, MT + 128], f32, kind="ExternalInput").ap()
    w2t = nc.dram_tensor("w2t", [128, MT], mybir.dt.bfloat16, kind="ExternalInput").ap()
    ctx_out = nc.dram_tensor("ctx_out", [BL, C], f32, kind="ExternalOutput").ap()
    attn_out = nc.dram_tensor("attn_out", [BL, HW], f32, kind="ExternalOutput").ap()

    with tile.TileContext(nc) as tc:
        with (
            tc.tile_pool(name="consts", bufs=1) as consts,
            tc.tile_pool(name="w1s_p", bufs=1) as w1s_p,
            tc.tile_pool(name="w1h_p", bufs=2) as w1h_p,
            tc.tile_pool(name="rhs_p", bufs=4) as rhs_p,
            tc.tile_pool(name="tanh_p", bufs=2) as tanh_p,
            tc.tile_pool(name="small_p", bufs=2) as small_p,
            tc.tile_pool(name="ps_gemm", bufs=4, space="PSUM") as ps_gemm,
            tc.tile_pool(name="ps_sc", bufs=2, space="PSUM") as ps_sc,
            tc.tile_pool(name="ps_aux", bufs=2, space="PSUM") as ps_aux,
        ):
            # ---------------- DMA order: unblock the GEMM ASAP -------------
            b1i_sb = consts.tile([128, MT + 128], f32)
            nc.sync.dma_start(b1i_sb[:], b1i)
            b1_sb = b1i_sb[:, :MT]
            ident = b1i_sb[:, MT:]
            w2_sb = consts.tile([128, MT], mybir.dt.bfloat16)
            nc.sync.dma_start(w2_sb[:], w2t)
            hid_sb = consts.tile([BL, HID], f32)
            nc.sync.dma_start(hid_sb[:], hidden)

            # All loads ride one HWDGE queue (FIFO = explicit bandwidth
            # schedule; two queues would split bandwidth 50/50 at packet
            # granularity and starve W1s during ramp-up). W1s goes m-major
            # so GEMM m-tiles unlock early; the first two rhs blocks
            # interleave between W1s m-tiles; later rhs DMAs are naturally
            # delayed by tile-slot reuse.

            def new_rhs(j):
                rhs = rhs_p.tile([128, KT, BPB, HW], bf16, name="rhs", tag="rhs")
                nc.sync.dma_start(
                    rhs[:],
                    spatial[:, j * BPB:(j + 1) * BPB, :]
                    .rearrange("(k p) b h -> p k b h", p=128))
                return rhs

            w1s_sb = w1s_p.tile([128, KT, HID], bf16)

            def dma_w1s_m(m, n=1):
                nc.sync.dma_start(
                    w1s_sb[:, :, m * 128:(m + n) * 128],
                    w1s[:, m * 128:(m + n) * 128]
                    .rearrange("(k p) d -> p k d", p=128))

            dma_w1s_m(0, 2)
            # block 0: per-k rhs DMAs so m0's k-loop starts on first arrival
            rhs0 = rhs_p.tile([128, KT, BPB, HW], bf16, name="rhs", tag="rhs")
            for kg in range(0, KT, 4):
                nc.sync.dma_start(
                    rhs0[:, kg:kg + 4, :, :],
                    spatial[kg * 128:(kg + 4) * 128, 0:BPB, :]
                    .rearrange("(k p) b h -> p k b h", p=128))
            w1h_tiles = []
            for k in range(HT):
                w1h_t = w1h_p.tile([128, HID], bf16, name="w1h_t", tag="w1h")
                nc.sync.dma_start(w1h_t[:], w1h[k * 128:(k + 1) * 128, :])
                w1h_tiles.append(w1h_t)
            dma_w1s_m(2, 2)
            rhs1 = new_rhs(1)
            dma_w1s_m(4, 2)
            dma_w1s_m(6, 2)

            # ---------------- per-batch bias: hb = (hidden @ W1h).T + b1 ---
            # transpose hidden -> hidT [h, b] (8 PE transposes of [32, 128])
            hidT_sb = consts.tile([128, HT, BL], bf16)
            for k in range(HT):
                ps_t = ps_aux.tile([128, BL], f32, tag="aux")
                nc.tensor.matmul(ps_t[:], hid_sb[:, k * 128:(k + 1) * 128],
                                 ident[:BL, :BL], is_transpose=True)
                nc.scalar.activation(hidT_sb[:, k, :], ps_t[:], AF.Identity)

            # hid1 = hidden @ W1h  [BL, HID]  (bf16)
            hid1_sb = consts.tile([BL, HID], f32)
            ps_h1 = [ps_aux.tile([BL, 512], f32, name=f"ps_h1_{c}", tag="aux")
                     for c in range(2)]
            for k in range(HT):
                for chunk in range(2):
                    nc.tensor.matmul(ps_h1[chunk][:], hidT_sb[:, k, :],
                                     w1h_tiles[k][:, chunk * 512:(chunk + 1) * 512],
                                     start=(k == 0), stop=(k == HT - 1))
            for chunk in range(2):
                nc.scalar.activation(hid1_sb[:, chunk * 512:(chunk + 1) * 512],
                                     ps_h1[chunk][:], AF.Identity)

            # hb[d, b] = hid1.T + b1  (8 transposes of [32, 128] + bias add)
            hb_sb = consts.tile([128, MT, BL], f32)
            for m in range(MT):
                ps_t = ps_aux.tile([128, BL], f32, tag="aux")
                nc.tensor.matmul(ps_t[:], hid1_sb[:, m * 128:(m + 1) * 128],
                                 ident[:BL, :BL], is_transpose=True)
                nc.scalar.activation(hb_sb[:, m, :], ps_t[:], AF.Identity,
                                     bias=b1_sb[:, m:m + 1])

            # context accumulator ctxT[c_lo, (k, b)]
            ctxT = consts.tile([128, KT * BL], f32)
            ctx_sb = consts.tile([BL, C], f32)

            # ---------------- pipelined epilogue ---------------------------
            def epilogue(j, ps_score, rhs, final=False):
                # softmax over hw within each batch segment (partition 0).
                # No max subtraction: |scores| <= ||W2||_1 ~ 9 (tanh bounded),
                # so exp cannot overflow and softmax is shift-invariant.
                e_sb = small_p.tile([1, BPB, HW], f32, name="e", tag="e")
                sm = small_p.tile([1, BPB, 1], f32, name="sm", tag="sm")
                # exp + per-segment sum fused on ScalarE via accum_out
                for bi in range(BPB):
                    nc.scalar.activation(e_sb[:, bi, :], ps_score[:, bi, :],
                                         AF.Exp, accum_out=sm[:, bi, :])
                # broadcast UNNORMALIZED exponentials right away (GpSimd);
                # the 1/sum normalization folds into the context stt's
                # per-partition scalar, keeping the normalize multiply off
                # the critical drain chain
                att_bc = small_p.tile([128, BPB, HW], f32, name="att_bc",
                                      tag="att_bc")
                nc.gpsimd.partition_broadcast(att_bc[:], e_sb[:])
                rec = small_p.tile([1, BPB, 1], f32, name="rec", tag="rec")
                nc.vector.reciprocal(rec[:], sm[:])
                rec_bc = small_p.tile([128, BPB], f32, name="rec_bc",
                                      tag="rec_bc")
                nc.gpsimd.partition_broadcast(rec_bc[:], rec[:])
                attn_sb = small_p.tile([1, BPB, HW], f32, name="attn", tag="attn")
                nc.vector.tensor_mul(attn_sb[:], e_sb[:],
                                     rec[:].broadcast_to([1, BPB, HW]))
                nc.sync.dma_start(attn_out[j * BPB:(j + 1) * BPB, :], attn_sb[:])

                # context: ctxT[c, (k, col)] = sum_hw rhs * attn (fused on DVE)
                junk = small_p.tile([128, HW], f32, name="junk", tag="junk")
                for k in range(KT):
                    for bi in range(BPB):
                        col = k * BL + j * BPB + bi
                        nc.vector.scalar_tensor_tensor(
                            out=junk[:],
                            in0=rhs[:, k, bi, :],
                            scalar=rec_bc[:, bi:bi + 1],
                            in1=att_bc[:, bi, :],
                            op0=mybir.AluOpType.mult,
                            op1=mybir.AluOpType.mult,
                            accum_out=ctxT[:, col:col + 1])
                    if final:
                        # ctxT row-k complete: transpose + stage + write out
                        # in quarters (single staging tile, no slot churn)
                        ps_t2 = ps_aux.tile([BL, 128], f32, tag="aux")
                        nc.tensor.matmul(ps_t2[:], ctxT[:, k * BL:(k + 1) * BL],
                                         ident[:, :], is_transpose=True)
                        nc.scalar.activation(
                            ctx_sb[:, k * 128:(k + 1) * 128], ps_t2[:],
                            AF.Identity)
                        nc.sync.dma_start(
                            ctx_out[:, k * 128:(k + 1) * 128],
                            ctx_sb[:, k * 128:(k + 1) * 128])

            # ---------------- main loop over column blocks -----------------
            pending = None
            for j in range(NBLK):
                rhs = (rhs0, rhs1)[j] if j < 2 else new_rhs(j)

                tanh_sb = tanh_p.tile([128, MT, BPB, HW], mybir.dt.bfloat16, name="tanh_sb",
                                      tag="tanh")
                for m in range(MT):
                    ps_pre = ps_gemm.tile([128, BPB, HW], f32, name="ps_pre",
                                          tag="gemm")
                    for k in range(KT):
                        nc.tensor.matmul(
                            ps_pre[:],
                            w1s_sb[:, k, m * 128:(m + 1) * 128],
                            rhs[:, k, :, :],
                            start=(k == 0), stop=(k == KT - 1))
                    for bi in range(BPB):
                        nc.scalar.activation(
                            tanh_sb[:, m, bi, :], ps_pre[:, bi, :], AF.Tanh,
                            bias=hb_sb[:, m, j * BPB + bi:j * BPB + bi + 1])

                # scores[(b, hw)] = sum_d tanh * w2
                ps_score = ps_sc.tile([1, BPB, HW], f32, name="ps_score",
                                      tag="score")
                for m in range(MT):
                    nc.tensor.matmul(ps_score[:], w2_sb[:, m:m + 1],
                                     tanh_sb[:, m, :, :],
                                     start=(m == 0), stop=(m == MT - 1))

                if pending is not None:
                    epilogue(*pending)
                pending = (j, ps_score, rhs)
            epilogue(*pending, final=True)

    nc.compile()
    return nc


def _get_nc():
    if "nc" not in _CACHE:
        _CACHE["nc"] = _build()
    return _CACHE["nc"]


def kernel(hidden, spatial_features, W1, b1, W2, b2):
    from concourse.bass_utils import run_bass_kernel_spmd

    hidden = np.asarray(hidden, dtype=np.float32)
    spatial = np.asarray(spatial_features, dtype=np.float32).reshape(B, C, HW)
    import ml_dtypes
    spatial_bf = spatial.astype(ml_dtypes.bfloat16)
    W1 = np.asarray(W1, dtype=np.float32)
    b1 = np.asarray(b1, dtype=np.float32)
    W2 = np.asarray(W2, dtype=np.float32)

    import ml_dtypes
    w1h = np.ascontiguousarray(W1[:HID].astype(ml_dtypes.bfloat16))
    w1s = np.ascontiguousarray(W1[HID:].astype(ml_dtypes.bfloat16))
    b1i = np.ascontiguousarray(np.concatenate(
        [b1.reshape(MT, 128).T, np.eye(128, dtype=np.float32)], axis=1))
    w2t = np.ascontiguousarray(W2[:, 0].reshape(MT, 128).T.astype(ml_dtypes.bfloat16))

    in_maps = [
        {
            "hidden": np.ascontiguousarray(hidden[i * BL:(i + 1) * BL]),
            "spatial": np.ascontiguousarray(
                spatial_bf[i * BL:(i + 1) * BL].transpose(1, 0, 2)),
            "w1h": w1h, "w1s": w1s, "b1i": b1i, "w2t": w2t,
        }
        for i in range(NCORES)
    ]
    nc = _get_nc()
    res = run_bass_kernel_spmd(nc, in_maps, core_ids=list(range(NCORES)))
    ctx = np.concatenate([res.results[i]["ctx_out"] for i in range(NCORES)], axis=0)
    attn = np.concatenate([res.results[i]["attn_out"] for i in range(NCORES)], axis=0)
    return ctx, attn
